# revision 56
# baseline (speedup 1.0000x reference)
"""BertAttention (B=8, S=1024, H=1024, 16 heads) on 8 TRN2 NeuronCores.

Strategy: data-parallel over batch -- core b computes batch element b
end-to-end (QKV proj, attention, output proj, residual, LayerNorm).
No collectives needed.

fp8 mode (default):
  - All four projections and the PV matmul run in fp8e4 with
    perf_mode=DoubleRow: contraction tiles are packed in pairs
    [128, 2, *], halving the matmul count. Weights are pre-scaled by
    16 on the host (fp8e4 dynamic range), descaled in the PSUM
    eviction op.
  - Scores stay bf16 (Q/K tiles written bf16): contraction is HD=64
    so the two heads of a pair run row-tiled (disjoint PE row groups).
  - Softmax exp runs on the scalar engine writing fp8 e-tiles directly;
    denominators come from a ones-column folded into V (PV row 64).
    ctx is scaled by 32 into fp8; the output projection descales by
    1/(16*32) when evicting PSUM.
  - Emission order interleaves projections with attention so the
    scalar engine's exp stream (the ~140us serial floor) starts early
    and overlaps all remaining PE work: Q0/K0 -> scores/exp(0) -> Q1/K1
    -> scores/exp(1) -> V-proj -> per-head loop (scores/exp t, Q/K t+1,
    PV t-1, O-partials as ctx pairs complete) -> final O partial fused
    with residual LayerNorm (stats via stt accum_out + ACT Square).

bf16 / f32r / f32 modes: earlier single-phase design kept for A/B.
"""

import sys

sys.path.insert(0, "/opt/trn_rl_repo")

import numpy as np

B, S, H = 8, 1024, 1024
NH, HD = 16, 64
LN_EPS = 1e-12
N_CORES = 8

MM_DTYPE = "fp8"  # "fp8" | "f32" | "f32r" | "bf16"

WSCALE = 16.0   # host pre-scale on all weight matrices (fp8 mode)
CSCALE = 32.0   # ctx pre-scale into fp8 before the output projection

PROJ_DR = True  # DoubleRow for QKVO projections
PV_DR = True    # DoubleRow for the PV matmul

_compiled = {}


def _build_fp8(n_reps=1, use_gb=True):
    import concourse.tile as tile
    from concourse import bacc, mybir

    F32 = mybir.dt.float32
    BF16 = mybir.dt.bfloat16
    FP8 = mybir.dt.float8e4
    AF = mybir.ActivationFunctionType
    ALU = mybir.AluOpType
    DR = mybir.MatmulPerfMode.DoubleRow

    nc = bacc.Bacc("TRN2", target_bir_lowering=False)

    # fp8 pair layouts: [4 pairs, 128 part, 2 interleave, cols]
    xtp_d = nc.dram_tensor("xtp", [4, 128, 2, S], FP8, kind="ExternalInput")
    wqp_d = nc.dram_tensor("wqp", [4, 128, 2, H], FP8, kind="ExternalInput")
    wkp_d = nc.dram_tensor("wkp", [4, 128, 2, H], FP8, kind="ExternalInput")
    wvp_d = nc.dram_tensor("wvp", [4, 128, 2, H], FP8, kind="ExternalInput")
    wop_d = nc.dram_tensor("wop", [4, 128, 2, H], FP8, kind="ExternalInput")
    xr_d = nc.dram_tensor("xr", [S, H], F32, kind="ExternalInput")
    bq_d = nc.dram_tensor("bq", [128, 8], F32, kind="ExternalInput")
    bk_d = nc.dram_tensor("bk", [128, 8], F32, kind="ExternalInput")
    bv_d = nc.dram_tensor("bv", [1, H], F32, kind="ExternalInput")
    mask_d = nc.dram_tensor("mask", [128, 8], F32, kind="ExternalInput")
    gamma_d = nc.dram_tensor("gamma", [1, H], F32, kind="ExternalInput")
    beta_d = nc.dram_tensor("beta", [1, H], F32, kind="ExternalInput")
    out_d = nc.dram_tensor("out", [S, H], F32, kind="ExternalOutput")

    NT = 8          # 128-row tiles per 1024 dim
    NP = 4          # contraction pair-tiles (2x128)
    NCH = 2         # 512-col chunks per 1024 dim
    CH = 512
    GW = NH * 65    # per-interleave vt row width (16 heads x (64 + ones))

    with tile.TileContext(nc) as tc:
      for _rep in range(n_reps):
        with (
            tc.tile_pool(name="consts", bufs=1) as cp,
            tc.tile_pool(name="xtp", bufs=1) as xtp_pool,
            tc.tile_pool(name="wp", bufs=4) as wp,
            tc.tile_pool(name="qt", bufs=8) as qt_pool,
            tc.tile_pool(name="kt", bufs=8) as kt_pool,
            tc.tile_pool(name="vt", bufs=4) as vt_pool,
            tc.tile_pool(name="ep", bufs=16) as ep,
            tc.tile_pool(name="ctx", bufs=4) as ctx_pool,
            tc.tile_pool(name="rp", bufs=4) as rp,
            tc.tile_pool(name="rbp", bufs=3) as rbp,
            tc.tile_pool(name="ob", bufs=8) as obp,
            tc.tile_pool(name="st", bufs=4) as stp,
            tc.tile_pool(name="pp", bufs=2, space="PSUM") as pp,
            tc.tile_pool(name="scps", bufs=2, space="PSUM") as scps,
            tc.tile_pool(name="cxps", bufs=2, space="PSUM") as cxps,
        ):
            # -------- input DMAs first (critical path to first exp) --------
            bq_sb = cp.tile([128, 8], F32)
            bk_sb = cp.tile([128, 8], F32)
            mask_sb = cp.tile([128, 8], F32)
            # split per-pair: 256KB transfers pipeline into the j-accumulation
            # (HBM bandwidth is shared -- one big DMA would gate the first MM
            # on the full 3MB)
            xtp_all = xtp_pool.tile([128, 4, 2, S], FP8, tag="xtp", name="xtp_t")
            wq_all = wp.tile([128, 4, 2, H], FP8, tag="w", name="wq_t")
            wk_all = wp.tile([128, 4, 2, H], FP8, tag="w", name="wk_t")
            for j in range(NP):
                nc.scalar.dma_start(out=xtp_all[:, j], in_=xtp_d[j])
                nc.sync.dma_start(out=wq_all[:, j], in_=wqp_d[j])
                nc.gpsimd.dma_start(out=wk_all[:, j], in_=wkp_d[j])
            xtp = [xtp_all[:, j] for j in range(NP)]
            wq_t = [wq_all[:, j] for j in range(NP)]
            wk_t = [wk_all[:, j] for j in range(NP)]
            nc.sync.dma_start(out=bq_sb, in_=bq_d[:])
            nc.sync.dma_start(out=bk_sb, in_=bk_d[:])
            nc.sync.dma_start(out=mask_sb, in_=mask_d[:])

            # ---------------- constants ----------------
            bv_sb = cp.tile([128, H], F32)
            gamma_sb = cp.tile([128, H], F32)
            beta_sb = cp.tile([128, H], F32)
            bv_row = cp.tile([1, H], F32)
            gamma_row = cp.tile([1, H], F32)
            beta_row = cp.tile([1, H], F32)
            eps_sb = cp.tile([128, 1], F32)
            nc.vector.memset(eps_sb[:], LN_EPS)

            # vt pair tiles + ones columns for softmax denominators
            vt = []
            for u in range(NP):
                v_t = vt_pool.tile([128, 2 * GW], FP8, tag="vt", name=f"vt{u}")
                nc.vector.memset(
                    v_t[:].rearrange("p (i g e) -> p i g e", i=2, e=65)
                    [:, :, :, 64:65], 1.0)
                vt.append(v_t)

            qt = [qt_pool.tile([128, S], BF16, tag="qt", name=f"qt{t}")
                  for t in range(NT)]
            kt = [kt_pool.tile([128, S], BF16, tag="kt", name=f"kt{t}")
                  for t in range(NT)]
            ctxt = [ctx_pool.tile([128, 2 * S], FP8, tag="ctx", name=f"ctx{u}")
                    for u in range(NP)]

            def pair_mm(ps, lhsT3, rhs3, first, last, dr):
                # lhsT3/rhs3: [128, 2, *] pair APs; DR or two plain fp8 MMs
                if dr:
                    nc.tensor.matmul(ps, lhsT=lhsT3, rhs=rhs3,
                                     start=first, stop=last, perf_mode=DR)
                else:
                    for i in range(2):
                        nc.tensor.matmul(
                            ps, lhsT=lhsT3[:, i:i + 1, :], rhs=rhs3[:, i:i + 1, :],
                            start=(first and i == 0), stop=(last and i == 1))

            def qk_proj(t):
                for n in range(NCH):
                    for w_tiles, b_sb, dst, eng in (
                        (wq_t, bq_sb, qt, nc.vector),
                        (wk_t, bk_sb, kt, nc.vector),
                    ):
                        ps = pp.tile([128, CH], F32, tag="pp", name="pp_t")
                        for j in range(NP):
                            pair_mm(
                                ps[:],
                                w_tiles[j][:, :, t * 128:(t + 1) * 128],
                                xtp[j][:, :, n * CH:(n + 1) * CH],
                                j == 0, j == NP - 1, PROJ_DR,
                            )
                        eng.tensor_scalar(
                            out=dst[t][:, n * CH:(n + 1) * CH], in0=ps[:],
                            scalar1=1.0 / WSCALE, scalar2=b_sb[:, t:t + 1],
                            op0=ALU.mult, op1=ALU.add,
                        )

            def scores_and_exp(t, e_tiles):
                for k in range(NT):
                    u, i = k // 2, k % 2
                    scs = [scps.tile([128, S], F32, tag="sc", name="sc_t")
                           for _ in range(2)]
                    for n in range(NCH):
                        for hh in range(2):
                            # adjacent emission on disjoint PE row groups
                            # (rows 0-63 / 64-127) -> concurrent row tiling
                            p0 = hh * 64
                            nc.tensor.matmul(
                                scs[hh][:, n * CH:(n + 1) * CH],
                                lhsT=kt[t][p0:p0 + 64, k * 128:(k + 1) * 128],
                                rhs=qt[t][p0:p0 + 64, n * CH:(n + 1) * CH],
                                start=True, stop=True,
                            )
                    for hh in range(2):
                        nc.scalar.activation(
                            e_tiles[(u, hh)][:, i * S:(i + 1) * S],
                            scs[hh][:], AF.Exp,
                            bias=mask_sb[:, k:k + 1], scale=0.125,
                        )

            def pv_and_norm(t, e_tiles):
                u_t, i_t = t // 2, t % 2
                for hh in range(2):
                    g = 2 * t + hh
                    for n in range(NCH):
                        cx = cxps.tile([65, CH], F32, tag="cx", name="cx_t")
                        for u in range(NP):
                            pair_mm(
                                cx[:],
                                vt[u][:]
                                .rearrange("p (i w) -> p i w", i=2)
                                [:, :, g * 65:(g + 1) * 65],
                                e_tiles[(u, hh)][:]
                                .rearrange("p (i q) -> p i q", i=2)
                                [:, :, n * CH:(n + 1) * CH],
                                u == 0, u == NP - 1, PV_DR,
                            )
                        recip = rp.tile([1, CH], F32, tag="recip", name="recip_t")
                        nc.vector.reciprocal(recip[:], cx[64:65, :])
                        rb = rbp.tile([64, CH], F32, tag="rb", name="rb_t")
                        nc.gpsimd.partition_broadcast(rb[:], recip[:])
                        nc.vector.scalar_tensor_tensor(
                            out=ctxt[u_t][hh * 64:(hh + 1) * 64,
                                          i_t * S + n * CH:i_t * S + (n + 1) * CH],
                            in0=cx[0:64, :], scalar=CSCALE, in1=rb[:],
                            op0=ALU.mult, op1=ALU.mult,
                        )

            # ---------------- pipeline ----------------
            e_tiles_all = {}
            for t in range(NT):
                e_tiles_all[t] = {
                    (u, hh): ep.tile([128, 2 * S], FP8, tag="e",
                                     name=f"e{t}_{u}_{hh}")
                    for u in range(NP) for hh in range(2)
                }
            # output accumulators: residual DMA'd in, O-proj u-partials
            # accumulated via DVE as each ctx pair completes
            o_tiles = []
            for mq in range(NT):
                o_t = obp.tile([128, H], F32, tag="ob", name=f"ob{mq}")
                o_tiles.append(o_t)

            def o_partial(us):
                # accumulate the given ctx pair tiles into o_tiles
                for mq in range(NT):
                    for n in range(NCH):
                        ps = pp.tile([128, CH], F32, tag="pp", name="pp_t")
                        for du, u in enumerate(us):
                            pair_mm(
                                ps[:],
                                ctxt[u][:]
                                .rearrange("p (i q) -> p i q", i=2)
                                [:, :, mq * 128:(mq + 1) * 128],
                                wo_t[u][:, :, n * CH:(n + 1) * CH],
                                du == 0, du == len(us) - 1, PROJ_DR,
                            )
                        nc.vector.scalar_tensor_tensor(
                            out=o_tiles[mq][:, n * CH:(n + 1) * CH],
                            in0=ps[:], scalar=1.0 / (WSCALE * CSCALE),
                            in1=o_tiles[mq][:, n * CH:(n + 1) * CH],
                            op0=ALU.mult, op1=ALU.add,
                        )

            qk_proj(0)
            scores_and_exp(0, e_tiles_all[0])
            qk_proj(1)

            # deferred consts (bv/gamma/beta broadcasts off the critical start)
            nc.sync.dma_start(out=bv_row, in_=bv_d[:])
            nc.sync.dma_start(out=gamma_row, in_=gamma_d[:])
            nc.sync.dma_start(out=beta_row, in_=beta_d[:])
            nc.gpsimd.partition_broadcast(bv_sb[:], bv_row[:])
            nc.gpsimd.partition_broadcast(gamma_sb[:], gamma_row[:])
            nc.gpsimd.partition_broadcast(beta_sb[:], beta_row[:])

            # residual loads + wo/wv weight loads (DMA idle here)
            for mq in range(NT):
                (nc.sync if mq % 2 == 0 else nc.gpsimd).dma_start(
                    out=o_tiles[mq], in_=xr_d[mq * 128:(mq + 1) * 128, :])
            wv_all = wp.tile([128, 4, 2, H], FP8, tag="w", name="wv_t")
            wo_all = wp.tile([128, 4, 2, H], FP8, tag="w", name="wo_t")
            for j in range(NP):
                (nc.sync if j % 2 == 0 else nc.gpsimd).dma_start(
                    out=wv_all[:, j], in_=wvp_d[j])
                nc.scalar.dma_start(out=wo_all[:, j], in_=wop_d[j])
            wv_t = [wv_all[:, j] for j in range(NP)]
            wo_t = [wo_all[:, j] for j in range(NP)]

            scores_and_exp(1, e_tiles_all[1])
            qk_proj(2)

            # V projection (overlaps exp(0)/exp(1) on ACT)
            for mk in range(NT):
                u, i = mk // 2, mk % 2
                for n in range(NCH):
                    ps = pp.tile([128, CH], F32, tag="pp", name="pp_t")
                    for j in range(NP):
                        pair_mm(
                            ps[:],
                            xtp[j][:, :, mk * 128:(mk + 1) * 128],
                            wv_t[j][:, :, n * CH:(n + 1) * CH],
                            j == 0, j == NP - 1, PROJ_DR,
                        )
                    nc.vector.scalar_tensor_tensor(
                        out=vt[u][:, i * GW + n * 8 * 65:i * GW + (n + 1) * 8 * 65]
                        .rearrange("p (g e) -> p g e", e=65)[:, :, 0:64],
                        in0=ps[:].rearrange("p (g e) -> p g e", e=64),
                        scalar=1.0 / WSCALE,
                        in1=bv_sb[:, n * CH:(n + 1) * CH]
                        .rearrange("p (g e) -> p g e", e=64),
                        op0=ALU.mult, op1=ALU.add,
                    )
            pv_and_norm(0, e_tiles_all[0])

            for t in range(2, NT):
                scores_and_exp(t, e_tiles_all[t])
                if t < NT - 1:
                    qk_proj(t + 1)
                pv_and_norm(t - 1, e_tiles_all[t - 1])
                if t == 4:
                    o_partial([0, 1])
                elif t == 6:
                    o_partial([2])
            pv_and_norm(NT - 1, e_tiles_all[NT - 1])

            # -------- final O partial fused with LayerNorm + store --------
            # LN stats without bn_stats: sum(x) free via the eviction stt's
            # accum_out, sum(x^2) via an ACT Square (ACT is idle in the tail).
            for mq in range(NT):
                o_t = o_tiles[mq]
                ps = scps.tile([128, S], F32, tag="sc", name="o3ps_t")
                for n in range(NCH):
                    pair_mm(
                        ps[:, n * CH:(n + 1) * CH],
                        ctxt[3][:]
                        .rearrange("p (i q) -> p i q", i=2)
                        [:, :, mq * 128:(mq + 1) * 128],
                        wo_t[3][:, :, n * CH:(n + 1) * CH],
                        True, True, PROJ_DR,
                    )
                xsum = stp.tile([128, 1], F32, tag="xsum", name="xsum_t")
                nc.vector.scalar_tensor_tensor(
                    out=o_t[:], in0=ps[:], scalar=1.0 / (WSCALE * CSCALE),
                    in1=o_t[:], op0=ALU.mult, op1=ALU.add,
                    accum_out=xsum[:],
                )
                sq_scr = stp.tile([128, H], F32, tag="sq", name="sq_t")
                xsq = stp.tile([128, 1], F32, tag="xsq", name="xsq_t")
                nc.scalar.activation(
                    sq_scr[:], o_t[:], AF.Square, accum_out=xsq[:],
                )
                # mu = xsum/H; var = xsq/H - mu^2; rstd = 1/sqrt(var+eps)
                mu = stp.tile([128, 1], F32, tag="mu", name="mu_t")
                nc.vector.tensor_scalar_mul(mu[:], xsum[:], 1.0 / H)
                mu2 = stp.tile([128, 1], F32, tag="mu2", name="mu2_t")
                nc.vector.tensor_mul(mu2[:], mu[:], mu[:])
                var = stp.tile([128, 1], F32, tag="var", name="var_t")
                nc.vector.scalar_tensor_tensor(
                    out=var[:], in0=xsq[:], scalar=1.0 / H, in1=mu2[:],
                    op0=ALU.mult, op1=ALU.subtract,
                )
                std = stp.tile([128, 1], F32, tag="std", name="std_t")
                nc.scalar.activation(std[:], var[:], AF.Sqrt, bias=eps_sb[:])
                rstd = stp.tile([128, 1], F32, tag="rstd", name="rstd_t")
                nc.vector.reciprocal(rstd[:], std[:])
                # (x - mu) * rstd as affine: rstd*x + (-mu*rstd)
                nmur = stp.tile([128, 1], F32, tag="nmur", name="nmur_t")
                nc.vector.tensor_scalar(
                    out=nmur[:], in0=mu[:], scalar1=rstd[:], scalar2=-1.0,
                    op0=ALU.mult, op1=ALU.mult,
                )
                if mq % 2 == 0:
                    nc.scalar.activation(
                        o_t[:], o_t[:], AF.Identity,
                        bias=nmur[:], scale=rstd[:],
                    )
                else:
                    nc.vector.tensor_scalar(
                        out=o_t[:], in0=o_t[:], scalar1=rstd[:], scalar2=nmur[:],
                        op0=ALU.mult, op1=ALU.add,
                    )
                if use_gb:
                    nc.vector.tensor_mul(o_t[:], o_t[:], gamma_sb[:])
                    nc.vector.tensor_add(o_t[:], o_t[:], beta_sb[:])
                (nc.gpsimd if mq % 2 == 0 else nc.sync).dma_start(
                    out=out_d[mq * 128:(mq + 1) * 128, :], in_=o_t
                )

    nc.compile()
    return nc


def _host_prep_fp8(hidden_states, attention_mask, Wq, bq, Wk, bk, Wv, bv,
                   Wo, bo, ln_gamma, ln_beta):
    import ml_dtypes

    f32 = np.float32
    fp8 = ml_dtypes.float8_e4m3
    hs = np.ascontiguousarray(hidden_states, dtype=f32)

    def wpairs(w):
        # W [out, in] -> W^T * WSCALE as [128, 4, 2, out] fp8 pair layout
        wt = np.asarray(w, f32).T * WSCALE
        return np.ascontiguousarray(
            wt.reshape(4, 2, 128, H).transpose(0, 2, 1, 3)).astype(fp8)

    wq_p, wk_p, wv_p, wo_p = wpairs(Wq), wpairs(Wk), wpairs(Wv), wpairs(Wo)
    bq_r = np.ascontiguousarray(np.asarray(bq, f32).reshape(8, 128).T)
    bk_r = np.ascontiguousarray(np.asarray(bk, f32).reshape(8, 128).T)
    bv_r = np.ascontiguousarray(np.asarray(bv, f32).reshape(1, H))
    gamma_r = np.ascontiguousarray(np.asarray(ln_gamma, f32).reshape(1, H))
    beta_r = np.ascontiguousarray(np.asarray(ln_beta, f32).reshape(1, H))
    bo_r = np.asarray(bo, f32)
    mask = np.asarray(attention_mask, f32).reshape(B, S)

    in_maps = []
    for b in range(B):
        xt = hs[b].T  # [H, S]
        xtp = np.ascontiguousarray(
            xt.reshape(4, 2, 128, S).transpose(0, 2, 1, 3)).astype(fp8)
        xr = np.ascontiguousarray(hs[b] + bo_r[None, :])
        mask_r = np.ascontiguousarray(mask[b].reshape(8, 128).T)
        in_maps.append({
            "xtp": xtp, "xr": xr,
            "wqp": wq_p, "wkp": wk_p, "wvp": wv_p, "wop": wo_p,
            "bq": bq_r, "bk": bk_r, "bv": bv_r,
            "mask": mask_r, "gamma": gamma_r, "beta": beta_r,
        })
    return in_maps


def _build(mm_dtype, n_reps=1, use_gb=True):
    if mm_dtype == "fp8":
        return _build_fp8(n_reps, use_gb)
    import concourse.tile as tile
    from concourse import bacc, mybir

    F32 = mybir.dt.float32
    AF = mybir.ActivationFunctionType
    ALU = mybir.AluOpType

    if mm_dtype == "f32":
        DT = F32
        DRAM_DT = F32
    elif mm_dtype == "f32r":
        DT = mybir.dt.float32r
        DRAM_DT = F32  # declare f32, bitcast APs at DMA time
    elif mm_dtype == "bf16":
        DT = mybir.dt.bfloat16
        DRAM_DT = mybir.dt.bfloat16
    else:
        raise ValueError(mm_dtype)

    def dma_in(out_ap, in_ap, eng=None):
        # DMA into a DT-typed tile; for f32r the DRAM side is f32 and we
        # bitcast the source AP (value-preserving; verified on HW).
        if eng is None:
            eng = nc.sync
        if mm_dtype == "f32r":
            in_ap = in_ap.bitcast(DT)
        eng.dma_start(out=out_ap, in_=in_ap)

    nc = bacc.Bacc("TRN2", target_bir_lowering=False)

    xt_d = nc.dram_tensor("xt", [H, S], DRAM_DT, kind="ExternalInput")
    xr_d = nc.dram_tensor("xr", [S, H], F32, kind="ExternalInput")
    wq_d = nc.dram_tensor("wq", [H, H], DRAM_DT, kind="ExternalInput")
    wk_d = nc.dram_tensor("wk", [H, H], DRAM_DT, kind="ExternalInput")
    wv_d = nc.dram_tensor("wv", [H, H], DRAM_DT, kind="ExternalInput")
    wo_d = nc.dram_tensor("wo", [H, H], DRAM_DT, kind="ExternalInput")
    bq_d = nc.dram_tensor("bq", [128, 8], F32, kind="ExternalInput")
    bk_d = nc.dram_tensor("bk", [128, 8], F32, kind="ExternalInput")
    bv_d = nc.dram_tensor("bv", [1, H], F32, kind="ExternalInput")
    mask_d = nc.dram_tensor("mask", [128, 8], F32, kind="ExternalInput")
    gamma_d = nc.dram_tensor("gamma", [1, H], F32, kind="ExternalInput")
    beta_d = nc.dram_tensor("beta", [1, H], F32, kind="ExternalInput")
    out_d = nc.dram_tensor("out", [S, H], F32, kind="ExternalOutput")

    NT = 8          # 128-row tiles per 1024 dim
    NCH = 2         # 512-col chunks per 1024 dim
    CH = 512

    with tile.TileContext(nc) as tc:
      for _rep in range(n_reps):
        with (
            tc.tile_pool(name="consts", bufs=1) as cp,
            tc.tile_pool(name="qt", bufs=8) as qt_pool,
            tc.tile_pool(name="kt", bufs=8) as kt_pool,
            tc.tile_pool(name="vt", bufs=8) as vt_pool,
        ):
            bq_sb = cp.tile([128, 8], F32)
            bk_sb = cp.tile([128, 8], F32)
            mask_sb = cp.tile([128, 8], F32)
            nc.sync.dma_start(out=bq_sb, in_=bq_d[:])
            nc.sync.dma_start(out=bk_sb, in_=bk_d[:])
            nc.sync.dma_start(out=mask_sb, in_=mask_d[:])
            bv_row = cp.tile([1, H], F32)
            gamma_row = cp.tile([1, H], F32)
            beta_row = cp.tile([1, H], F32)
            nc.sync.dma_start(out=bv_row, in_=bv_d[:])
            nc.sync.dma_start(out=gamma_row, in_=gamma_d[:])
            nc.sync.dma_start(out=beta_row, in_=beta_d[:])
            bv_sb = cp.tile([128, H], F32)
            gamma_sb = cp.tile([128, H], F32)
            beta_sb = cp.tile([128, H], F32)
            nc.gpsimd.partition_broadcast(bv_sb[:], bv_row[:])
            nc.gpsimd.partition_broadcast(gamma_sb[:], gamma_row[:])
            nc.gpsimd.partition_broadcast(beta_sb[:], beta_row[:])
            eps_sb = cp.tile([128, 1], F32)
            nc.vector.memset(eps_sb[:], LN_EPS)
            ones_sb = cp.tile([128, NH], F32)
            nc.vector.memset(ones_sb[:], 1.0)

            qt = [qt_pool.tile([128, S], DT, tag="qt", name=f"qt{t}") for t in range(NT)]
            kt = [kt_pool.tile([128, S], DT, tag="kt", name=f"kt{t}") for t in range(NT)]
            # v tiles: per k-tile, 16 heads x (64 v-cols + ones col)
            vt = [vt_pool.tile([128, NH * 65], DT, tag="vt", name=f"vt{t}") for t in range(NT)]

            # ---------------- QKV projections ----------------
            with (
                tc.tile_pool(name="xt", bufs=8) as xt_pool,
                tc.tile_pool(name="wp", bufs=13) as wp,
                tc.tile_pool(name="pp", bufs=4, space="PSUM") as pp,
            ):
                xt = []
                for t in range(NT):
                    x_t = xt_pool.tile([128, S], DT, tag="xt", name=f"xt{t}")
                    dma_in(x_t, xt_d[t * 128:(t + 1) * 128, :],
                           eng=(nc.scalar if t % 2 == 0 else nc.gpsimd))
                    xt.append(x_t)

                # V projection: natural [k, dv] layout; lhsT = XT tiles.
                wv_tiles = []
                for t in range(NT):
                    w_t = wp.tile([128, H], DT, tag="w", name=f"w_v{t}")
                    dma_in(w_t, wv_d[t * 128:(t + 1) * 128, :],
                           eng=(nc.sync if t % 2 == 0 else nc.scalar))
                    wv_tiles.append(w_t)
                for mk in range(NT):
                    # ones columns for the softmax-denominator rows
                    nc.vector.tensor_copy(
                        vt[mk][:].rearrange("p (g e) -> p g e", e=65)[:, :, 64:65],
                        ones_sb[:].rearrange("p (g e) -> p g e", e=1),
                    )
                    for n in range(NCH):
                        ps = pp.tile([128, CH], F32, tag="pp", name="pp_t")
                        for h in range(NT):
                            nc.tensor.matmul(
                                ps[:],
                                lhsT=xt[h][:, mk * 128:(mk + 1) * 128],
                                rhs=wv_tiles[h][:, n * CH:(n + 1) * CH],
                                start=(h == 0),
                                stop=(h == NT - 1),
                            )
                        nc.vector.tensor_add(
                            vt[mk][:, n * 8 * 65:(n + 1) * 8 * 65]
                            .rearrange("p (g e) -> p g e", e=65)[:, :, 0:64],
                            ps[:].rearrange("p (g e) -> p g e", e=64),
                            bv_sb[:, n * CH:(n + 1) * CH]
                            .rearrange("p (g e) -> p g e", e=64),
                        )

                for name, w_dram, b_sb, dst in (
                    ("q", wq_d, bq_sb, qt),
                    ("k", wk_d, bk_sb, kt),
                ):
                    w_tiles = []
                    for t in range(NT):
                        w_t = wp.tile([128, H], DT, tag="w", name=f"w_{name}{t}")
                        dma_in(w_t, w_dram[t * 128:(t + 1) * 128, :])
                        w_tiles.append(w_t)
                    for m in range(NT):
                        for n in range(NCH):
                            ps = pp.tile([128, CH], F32, tag="pp", name="pp_t")
                            for h in range(NT):
                                nc.tensor.matmul(
                                    ps[:],
                                    lhsT=w_tiles[h][:, m * 128:(m + 1) * 128],
                                    rhs=xt[h][:, n * CH:(n + 1) * CH],
                                    start=(h == 0),
                                    stop=(h == NT - 1),
                                )
                            nc.vector.tensor_scalar_add(
                                dst[m][:, n * CH:(n + 1) * CH], ps[:],
                                b_sb[:, m:m + 1],
                            )

            # ---------------- attention (per head pair) ----------------
            with (
                tc.tile_pool(name="ep", bufs=12) as ep,
                tc.tile_pool(name="rp", bufs=4) as rp,
                tc.tile_pool(name="rbp", bufs=3) as rbp,
                tc.tile_pool(name="scps", bufs=2, space="PSUM") as scps,
                tc.tile_pool(name="cxps", bufs=4, space="PSUM") as cxps,
            ):
                ctxt = []
                for t in range(NT):  # head pair t = heads 2t, 2t+1
                    ctx_t = qt_pool.tile([128, S], DT, tag="qt", name=f"ctx{t}")
                    ctxt.append(ctx_t)
                    # 4 live PV accumulators: (head, chunk)
                    cxs = [[cxps.tile([65, CH], F32, tag="cx", name="cx_t")
                            for _ in range(NCH)] for _ in range(2)]
                    for k in range(NT):
                        # one [128, S] score psum per head per k-tile (2 banks);
                        # the two q-chunks fill its halves; one exp covers both.
                        # Head A (rows 0-63) and head B (rows 64-127) matmuls are
                        # emitted adjacently per chunk: disjoint PE row groups run
                        # concurrently (row tiling).
                        scs = []
                        for hh in range(2):
                            sc = scps.tile([128, S], F32, tag="sc", name="sc_t")
                            scs.append(sc)
                        for n in range(NCH):
                            for hh in range(2):
                                p0 = hh * 64
                                nc.tensor.matmul(
                                    scs[hh][:, n * CH:(n + 1) * CH],
                                    lhsT=kt[t][p0:p0 + 64, k * 128:(k + 1) * 128],
                                    rhs=qt[t][p0:p0 + 64, n * CH:(n + 1) * CH],
                                    start=True,
                                    stop=True,
                                )
                        for hh in range(2):
                            g = 2 * t + hh
                            e_t = ep.tile([128, S], DT, tag="e", name="e_t")
                            nc.scalar.activation(
                                e_t[:], scs[hh][:], AF.Exp,
                                bias=mask_sb[:, k:k + 1], scale=0.125,
                            )
                            for n in range(NCH):
                                nc.tensor.matmul(
                                    cxs[hh][n][:],
                                    lhsT=vt[k][:, g * 65:(g + 1) * 65],
                                    rhs=e_t[:, n * CH:(n + 1) * CH],
                                    start=(k == 0),
                                    stop=(k == NT - 1),
                                )
                    for hh in range(2):
                        for n in range(NCH):
                            cx = cxs[hh][n]
                            recip = rp.tile([1, CH], F32, tag="recip", name="recip_t")
                            nc.vector.reciprocal(recip[:], cx[64:65, :])
                            rb = rbp.tile([64, CH], F32, tag="rb", name="rb_t")
                            nc.gpsimd.partition_broadcast(rb[:], recip[:])
                            nc.vector.tensor_mul(
                                ctx_t[hh * 64:hh * 64 + 64, n * CH:(n + 1) * CH],
                                cx[0:64, :],
                                rb[:],
                            )

            # ---------------- output proj + residual + LayerNorm ----------------
            with (
                tc.tile_pool(name="wo", bufs=8) as wop,
                tc.tile_pool(name="xr", bufs=5) as xrp,
                tc.tile_pool(name="ob", bufs=4) as obp,
                tc.tile_pool(name="st", bufs=4) as stp,
                tc.tile_pool(name="po", bufs=4, space="PSUM") as po,
            ):
                wo_tiles = []
                for t in range(NT):
                    w_t = wop.tile([128, H], DT, tag="wo", name=f"wo{t}")
                    dma_in(w_t, wo_d[t * 128:(t + 1) * 128, :])
                    wo_tiles.append(w_t)
                for mq in range(NT):
                    xr_t = xrp.tile([128, H], F32, tag="xr", name="xr_t")
                    (nc.sync if mq % 2 == 0 else nc.gpsimd).dma_start(
                        out=xr_t, in_=xr_d[mq * 128:(mq + 1) * 128, :]
                    )
                    o_t = obp.tile([128, H], F32, tag="ob", name="ob_t")
                    for n in range(NCH):
                        ps = po.tile([128, CH], F32, tag="po", name="po_t")
                        for t in range(NT):
                            nc.tensor.matmul(
                                ps[:],
                                lhsT=ctxt[t][:, mq * 128:(mq + 1) * 128],
                                rhs=wo_tiles[t][:, n * CH:(n + 1) * CH],
                                start=(t == 0),
                                stop=(t == NT - 1),
                            )
                        nc.vector.tensor_add(
                            o_t[:, n * CH:(n + 1) * CH], ps[:],
                            xr_t[:, n * CH:(n + 1) * CH],
                        )
                    stats = stp.tile([128, 2, 6], F32, tag="stats", name="stats_t")
                    for sg in range(2):
                        nc.vector.bn_stats(
                            stats[:, sg, :], o_t[:, sg * CH:(sg + 1) * CH]
                        )
                    mv = stp.tile([128, 2], F32, tag="mv", name="mv_t")
                    nc.vector.bn_aggr(mv[:], stats[:])
                    mu = mv[:, 0:1]
                    var = mv[:, 1:2]
                    std = stp.tile([128, 1], F32, tag="std", name="std_t")
                    nc.scalar.activation(std[:], var[:], AF.Sqrt, bias=eps_sb[:])
                    rstd = stp.tile([128, 1], F32, tag="rstd", name="rstd_t")
                    nc.vector.reciprocal(rstd[:], std[:])
                    # (x - mu) * rstd as ACT affine: rstd*x + (-mu*rstd)
                    nmur = stp.tile([128, 1], F32, tag="nmur", name="nmur_t")
                    nc.vector.tensor_scalar(
                        out=nmur[:], in0=mu, scalar1=rstd[:], scalar2=-1.0,
                        op0=ALU.mult, op1=ALU.mult,
                    )
                    nc.scalar.activation(
                        o_t[:], o_t[:], AF.Identity,
                        bias=nmur[:], scale=rstd[:],
                    )
                    if use_gb:
                        nc.vector.tensor_mul(o_t[:], o_t[:], gamma_sb[:])
                        nc.vector.tensor_add(o_t[:], o_t[:], beta_sb[:])
                    (nc.gpsimd if mq % 2 == 0 else nc.sync).dma_start(
                        out=out_d[mq * 128:(mq + 1) * 128, :], in_=o_t
                    )

    nc.compile()
    return nc


def _host_prep(mm_dtype, hidden_states, attention_mask, Wq, bq, Wk, bk, Wv, bv,
               Wo, bo, ln_gamma, ln_beta):
    if mm_dtype == "fp8":
        return _host_prep_fp8(hidden_states, attention_mask, Wq, bq, Wk, bk,
                              Wv, bv, Wo, bo, ln_gamma, ln_beta)
    f32 = np.float32
    hs = np.ascontiguousarray(hidden_states, dtype=f32)
    if mm_dtype == "bf16":
        import ml_dtypes
        wdt = ml_dtypes.bfloat16
    else:
        wdt = f32
    wqT = np.ascontiguousarray(np.asarray(Wq, dtype=f32).T).astype(wdt)
    wkT = np.ascontiguousarray(np.asarray(Wk, dtype=f32).T).astype(wdt)
    wvT = np.ascontiguousarray(np.asarray(Wv, dtype=f32).T).astype(wdt)
    woT = np.ascontiguousarray(np.asarray(Wo, dtype=f32).T).astype(wdt)
    bq_r = np.ascontiguousarray(np.asarray(bq, f32).reshape(8, 128).T)
    bk_r = np.ascontiguousarray(np.asarray(bk, f32).reshape(8, 128).T)
    bv_r = np.ascontiguousarray(np.asarray(bv, f32).reshape(1, H))
    gamma_r = np.ascontiguousarray(np.asarray(ln_gamma, f32).reshape(1, H))
    beta_r = np.ascontiguousarray(np.asarray(ln_beta, f32).reshape(1, H))
    bo_r = np.asarray(bo, f32)
    mask = np.asarray(attention_mask, f32).reshape(B, S)

    in_maps = []
    for b in range(B):
        xt = np.ascontiguousarray(hs[b].T).astype(wdt)
        xr = np.ascontiguousarray(hs[b] + bo_r[None, :])
        mask_r = np.ascontiguousarray(mask[b].reshape(8, 128).T)
        in_maps.append({
            "xt": xt, "xr": xr,
            "wq": wqT, "wk": wkT, "wv": wvT, "wo": woT,
            "bq": bq_r, "bk": bk_r, "bv": bv_r,
            "mask": mask_r, "gamma": gamma_r, "beta": beta_r,
        })
    return in_maps


def get_nc(mm_dtype=MM_DTYPE, n_reps=1, use_gb=True):
    key = (mm_dtype, n_reps, use_gb)
    if key not in _compiled:
        _compiled[key] = _build(mm_dtype, n_reps, use_gb)
    return _compiled[key]


def kernel(hidden_states, attention_mask, Wq, bq, Wk, bk, Wv, bv, Wo, bo,
           ln_gamma, ln_beta):
    from concourse.bass_utils import run_bass_kernel_spmd

    use_gb = not (
        np.all(np.asarray(ln_gamma) == 1.0) and np.all(np.asarray(ln_beta) == 0.0)
    )
    nc = get_nc(MM_DTYPE, use_gb=use_gb)
    in_maps = _host_prep(MM_DTYPE, hidden_states, attention_mask, Wq, bq,
                         Wk, bk, Wv, bv, Wo, bo, ln_gamma, ln_beta)
    res = run_bass_kernel_spmd(nc, in_maps, list(range(N_CORES)))
    out = np.stack([np.asarray(res.results[i]["out"]) for i in range(N_CORES)])
    return out.astype(np.float32)


# revision 66
# speedup vs baseline: 1.7045x; 1.7045x over previous
"""BertAttention (B=8, S=1024, H=1024, 16 heads) on 8 TRN2 NeuronCores.

Strategy: data-parallel over batch -- core b computes batch element b
end-to-end (QKV proj, attention, output proj, residual, LayerNorm).
No collectives needed.

fp8 mode (default):
  - All four projections and the PV matmul run in fp8e4 with
    perf_mode=DoubleRow: contraction tiles are packed in pairs
    [128, 2, *], halving the matmul count. Weights are pre-scaled by
    16 on the host (fp8e4 dynamic range), descaled in the PSUM
    eviction op.
  - Scores stay bf16 (Q/K tiles written bf16): contraction is HD=64
    so the two heads of a pair run row-tiled (disjoint PE row groups).
  - Softmax exp runs on the scalar engine writing fp8 e-tiles directly;
    denominators come from a ones-column folded into V (PV row 64).
    ctx is scaled by 32 into fp8; the output projection descales by
    1/(16*32) when evicting PSUM.
  - Emission order interleaves projections with attention so the
    scalar engine's exp stream (the ~140us serial floor) starts early
    and overlaps all remaining PE work: Q0/K0 -> scores/exp(0) -> Q1/K1
    -> scores/exp(1) -> V-proj -> per-head loop (scores/exp t, Q/K t+1,
    PV t-1, O-partials as ctx pairs complete) -> final O partial fused
    with residual LayerNorm (stats via stt accum_out + ACT Square).

bf16 / f32r / f32 modes: earlier single-phase design kept for A/B.
"""

import sys

sys.path.insert(0, "/opt/trn_rl_repo")

import numpy as np

B, S, H = 8, 1024, 1024
NH, HD = 16, 64
LN_EPS = 1e-12
N_CORES = 8

MM_DTYPE = "fp8"  # "fp8" | "f32" | "f32r" | "bf16"

WSCALE = 16.0   # host pre-scale on all weight matrices (fp8 mode)
CSCALE = 32.0   # ctx pre-scale into fp8 before the output projection

PROJ_DR = True  # DoubleRow for QKVO projections
PV_DR = True    # DoubleRow for the PV matmul

_compiled = {}


def _build_fp8(n_reps=1, use_gb=True):
    import concourse.tile as tile
    from concourse import bacc, mybir

    F32 = mybir.dt.float32
    BF16 = mybir.dt.bfloat16
    FP8 = mybir.dt.float8e4
    AF = mybir.ActivationFunctionType
    ALU = mybir.AluOpType
    DR = mybir.MatmulPerfMode.DoubleRow

    nc = bacc.Bacc("TRN2", target_bir_lowering=False)

    # fp8 pair layouts: [4 pairs, 128 part, 2 interleave, cols]
    xtp_d = nc.dram_tensor("xtp", [4, 128, 2, S], FP8, kind="ExternalInput")
    wqp_d = nc.dram_tensor("wqp", [4, 128, 2, H], FP8, kind="ExternalInput")
    wkp_d = nc.dram_tensor("wkp", [4, 128, 2, H], FP8, kind="ExternalInput")
    wvp_d = nc.dram_tensor("wvp", [4, 128, 2, H], FP8, kind="ExternalInput")
    wop_d = nc.dram_tensor("wop", [4, 128, 2, H], FP8, kind="ExternalInput")
    xr_d = nc.dram_tensor("xr", [S, H], F32, kind="ExternalInput")
    bq_d = nc.dram_tensor("bq", [128, 8], F32, kind="ExternalInput")
    bk_d = nc.dram_tensor("bk", [128, 8], F32, kind="ExternalInput")
    bv_d = nc.dram_tensor("bv", [1, H], F32, kind="ExternalInput")
    mask_d = nc.dram_tensor("mask", [128, 8], F32, kind="ExternalInput")
    gamma_d = nc.dram_tensor("gamma", [1, H], F32, kind="ExternalInput")
    beta_d = nc.dram_tensor("beta", [1, H], F32, kind="ExternalInput")
    out_d = nc.dram_tensor("out", [S, H], F32, kind="ExternalOutput")

    NT = 8          # 128-row tiles per 1024 dim
    NP = 4          # contraction pair-tiles (2x128)
    NCH = 2         # 512-col chunks per 1024 dim
    CH = 512
    GW = NH * 65    # per-interleave vt row width (16 heads x (64 + ones))

    with tile.TileContext(nc) as tc:
      for _rep in range(n_reps):
        with (
            tc.tile_pool(name="consts", bufs=1) as cp,
            tc.tile_pool(name="xtp", bufs=1) as xtp_pool,
            tc.tile_pool(name="wp", bufs=4) as wp,
            tc.tile_pool(name="qt", bufs=8) as qt_pool,
            tc.tile_pool(name="kt", bufs=8) as kt_pool,
            tc.tile_pool(name="vt", bufs=4) as vt_pool,
            tc.tile_pool(name="ep", bufs=16) as ep,
            tc.tile_pool(name="ctx", bufs=4) as ctx_pool,
            tc.tile_pool(name="rp", bufs=4) as rp,
            tc.tile_pool(name="rbp", bufs=3) as rbp,
            tc.tile_pool(name="ob", bufs=8) as obp,
            tc.tile_pool(name="st", bufs=4) as stp,
            tc.tile_pool(name="pp", bufs=2, space="PSUM") as pp,
            tc.tile_pool(name="scps", bufs=2, space="PSUM") as scps,
            tc.tile_pool(name="cxps", bufs=2, space="PSUM") as cxps,
        ):
            # -------- input DMAs first (critical path to first exp) --------
            bq_sb = cp.tile([128, 8], F32)
            bk_sb = cp.tile([128, 8], F32)
            mask_sb = cp.tile([128, 8], F32)
            # split per-pair: 256KB transfers pipeline into the j-accumulation
            # (HBM bandwidth is shared -- one big DMA would gate the first MM
            # on the full 3MB)
            xtp_all = xtp_pool.tile([128, 4, 2, S], FP8, tag="xtp", name="xtp_t")
            wq_all = wp.tile([128, 4, 2, H], FP8, tag="w", name="wq_t")
            wk_all = wp.tile([128, 4, 2, H], FP8, tag="w", name="wk_t")
            for j in range(NP):
                nc.scalar.dma_start(out=xtp_all[:, j], in_=xtp_d[j])
                nc.sync.dma_start(out=wq_all[:, j], in_=wqp_d[j])
                nc.gpsimd.dma_start(out=wk_all[:, j], in_=wkp_d[j])
            xtp = [xtp_all[:, j] for j in range(NP)]
            wq_t = [wq_all[:, j] for j in range(NP)]
            wk_t = [wk_all[:, j] for j in range(NP)]
            nc.sync.dma_start(out=bq_sb, in_=bq_d[:])
            nc.sync.dma_start(out=bk_sb, in_=bk_d[:])
            nc.sync.dma_start(out=mask_sb, in_=mask_d[:])

            # ---------------- constants ----------------
            bv_sb = cp.tile([128, H], F32)
            gamma_sb = cp.tile([128, H], F32)
            beta_sb = cp.tile([128, H], F32)
            bv_row = cp.tile([1, H], F32)
            gamma_row = cp.tile([1, H], F32)
            beta_row = cp.tile([1, H], F32)
            eps_sb = cp.tile([128, 1], F32)
            nc.vector.memset(eps_sb[:], LN_EPS)

            # vt pair tiles + ones columns for softmax denominators
            vt = []
            for u in range(NP):
                v_t = vt_pool.tile([128, 2 * GW], FP8, tag="vt", name=f"vt{u}")
                nc.vector.memset(
                    v_t[:].rearrange("p (i g e) -> p i g e", i=2, e=65)
                    [:, :, :, 64:65], 1.0)
                vt.append(v_t)

            qt = [qt_pool.tile([128, S], BF16, tag="qt", name=f"qt{t}")
                  for t in range(NT)]
            kt = [kt_pool.tile([128, S], BF16, tag="kt", name=f"kt{t}")
                  for t in range(NT)]
            ctxt = [ctx_pool.tile([128, 2 * S], FP8, tag="ctx", name=f"ctx{u}")
                    for u in range(NP)]

            def pair_mm(ps, lhsT3, rhs3, first, last, dr):
                # lhsT3/rhs3: [128, 2, *] pair APs; DR or two plain fp8 MMs
                if dr:
                    nc.tensor.matmul(ps, lhsT=lhsT3, rhs=rhs3,
                                     start=first, stop=last, perf_mode=DR)
                else:
                    for i in range(2):
                        nc.tensor.matmul(
                            ps, lhsT=lhsT3[:, i:i + 1, :], rhs=rhs3[:, i:i + 1, :],
                            start=(first and i == 0), stop=(last and i == 1))

            def qk_proj(t):
                for n in range(NCH):
                    for w_tiles, b_sb, dst, eng in (
                        (wq_t, bq_sb, qt, nc.vector),
                        (wk_t, bk_sb, kt, nc.vector),
                    ):
                        ps = pp.tile([128, CH], F32, tag="pp", name="pp_t")
                        for j in range(NP):
                            pair_mm(
                                ps[:],
                                w_tiles[j][:, :, t * 128:(t + 1) * 128],
                                xtp[j][:, :, n * CH:(n + 1) * CH],
                                j == 0, j == NP - 1, PROJ_DR,
                            )
                        eng.tensor_scalar(
                            out=dst[t][:, n * CH:(n + 1) * CH], in0=ps[:],
                            scalar1=1.0 / WSCALE, scalar2=b_sb[:, t:t + 1],
                            op0=ALU.mult, op1=ALU.add,
                        )

            def scores_and_exp(t, e_tiles):
                for k in range(NT):
                    u, i = k // 2, k % 2
                    scs = [scps.tile([128, S], F32, tag="sc", name="sc_t")
                           for _ in range(2)]
                    for n in range(NCH):
                        for hh in range(2):
                            # adjacent emission on disjoint PE row groups
                            # (rows 0-63 / 64-127) -> concurrent row tiling
                            p0 = hh * 64
                            nc.tensor.matmul(
                                scs[hh][:, n * CH:(n + 1) * CH],
                                lhsT=kt[t][p0:p0 + 64, k * 128:(k + 1) * 128],
                                rhs=qt[t][p0:p0 + 64, n * CH:(n + 1) * CH],
                                start=True, stop=True,
                            )
                    for hh in range(2):
                        nc.scalar.activation(
                            e_tiles[(u, hh)][:, i * S:(i + 1) * S],
                            scs[hh][:], AF.Exp,
                            bias=mask_sb[:, k:k + 1], scale=0.125,
                        )

            def pv_and_norm(t, e_tiles):
                u_t, i_t = t // 2, t % 2
                for hh in range(2):
                    g = 2 * t + hh
                    for n in range(NCH):
                        cx = cxps.tile([65, CH], F32, tag="cx", name="cx_t")
                        for u in range(NP):
                            pair_mm(
                                cx[:],
                                vt[u][:]
                                .rearrange("p (i w) -> p i w", i=2)
                                [:, :, g * 65:(g + 1) * 65],
                                e_tiles[(u, hh)][:]
                                .rearrange("p (i q) -> p i q", i=2)
                                [:, :, n * CH:(n + 1) * CH],
                                u == 0, u == NP - 1, PV_DR,
                            )
                        recip = rp.tile([1, CH], F32, tag="recip", name="recip_t")
                        nc.vector.reciprocal(recip[:], cx[64:65, :])
                        rb = rbp.tile([64, CH], F32, tag="rb", name="rb_t")
                        nc.gpsimd.partition_broadcast(rb[:], recip[:])
                        nc.vector.scalar_tensor_tensor(
                            out=ctxt[u_t][hh * 64:(hh + 1) * 64,
                                          i_t * S + n * CH:i_t * S + (n + 1) * CH],
                            in0=cx[0:64, :], scalar=CSCALE, in1=rb[:],
                            op0=ALU.mult, op1=ALU.mult,
                        )

            # ---------------- pipeline ----------------
            e_tiles_all = {}
            for t in range(NT):
                e_tiles_all[t] = {
                    (u, hh): ep.tile([128, 2 * S], FP8, tag="e",
                                     name=f"e{t}_{u}_{hh}")
                    for u in range(NP) for hh in range(2)
                }
            # output accumulators: residual DMA'd in, O-proj u-partials
            # accumulated via DVE as each ctx pair completes
            o_tiles = []
            for mq in range(NT):
                o_t = obp.tile([128, H], F32, tag="ob", name=f"ob{mq}")
                o_tiles.append(o_t)

            def o_partial(us):
                # accumulate the given ctx pair tiles into o_tiles
                for mq in range(NT):
                    for n in range(NCH):
                        ps = pp.tile([128, CH], F32, tag="pp", name="pp_t")
                        for du, u in enumerate(us):
                            pair_mm(
                                ps[:],
                                ctxt[u][:]
                                .rearrange("p (i q) -> p i q", i=2)
                                [:, :, mq * 128:(mq + 1) * 128],
                                wo_t[u][:, :, n * CH:(n + 1) * CH],
                                du == 0, du == len(us) - 1, PROJ_DR,
                            )
                        nc.vector.scalar_tensor_tensor(
                            out=o_tiles[mq][:, n * CH:(n + 1) * CH],
                            in0=ps[:], scalar=1.0 / (WSCALE * CSCALE),
                            in1=o_tiles[mq][:, n * CH:(n + 1) * CH],
                            op0=ALU.mult, op1=ALU.add,
                        )

            qk_proj(0)
            scores_and_exp(0, e_tiles_all[0])
            qk_proj(1)

            # deferred consts (bv/gamma/beta broadcasts off the critical start)
            nc.sync.dma_start(out=bv_row, in_=bv_d[:])
            nc.sync.dma_start(out=gamma_row, in_=gamma_d[:])
            nc.sync.dma_start(out=beta_row, in_=beta_d[:])
            nc.gpsimd.partition_broadcast(bv_sb[:], bv_row[:])
            nc.gpsimd.partition_broadcast(gamma_sb[:], gamma_row[:])
            nc.gpsimd.partition_broadcast(beta_sb[:], beta_row[:])

            # residual loads + wo/wv weight loads (DMA idle here)
            for mq in range(NT):
                (nc.sync if mq % 2 == 0 else nc.gpsimd).dma_start(
                    out=o_tiles[mq], in_=xr_d[mq * 128:(mq + 1) * 128, :])
            wv_all = wp.tile([128, 4, 2, H], FP8, tag="w", name="wv_t")
            wo_all = wp.tile([128, 4, 2, H], FP8, tag="w", name="wo_t")
            for j in range(NP):
                (nc.sync if j % 2 == 0 else nc.gpsimd).dma_start(
                    out=wv_all[:, j], in_=wvp_d[j])
                nc.scalar.dma_start(out=wo_all[:, j], in_=wop_d[j])
            wv_t = [wv_all[:, j] for j in range(NP)]
            wo_t = [wo_all[:, j] for j in range(NP)]

            scores_and_exp(1, e_tiles_all[1])
            qk_proj(2)

            # V projection (overlaps exp(0)/exp(1) on ACT)
            for mk in range(NT):
                u, i = mk // 2, mk % 2
                for n in range(NCH):
                    ps = pp.tile([128, CH], F32, tag="pp", name="pp_t")
                    for j in range(NP):
                        pair_mm(
                            ps[:],
                            xtp[j][:, :, mk * 128:(mk + 1) * 128],
                            wv_t[j][:, :, n * CH:(n + 1) * CH],
                            j == 0, j == NP - 1, PROJ_DR,
                        )
                    nc.vector.scalar_tensor_tensor(
                        out=vt[u][:, i * GW + n * 8 * 65:i * GW + (n + 1) * 8 * 65]
                        .rearrange("p (g e) -> p g e", e=65)[:, :, 0:64],
                        in0=ps[:].rearrange("p (g e) -> p g e", e=64),
                        scalar=1.0 / WSCALE,
                        in1=bv_sb[:, n * CH:(n + 1) * CH]
                        .rearrange("p (g e) -> p g e", e=64),
                        op0=ALU.mult, op1=ALU.add,
                    )
            pv_and_norm(0, e_tiles_all[0])

            for t in range(2, NT):
                scores_and_exp(t, e_tiles_all[t])
                if t < NT - 1:
                    qk_proj(t + 1)
                pv_and_norm(t - 1, e_tiles_all[t - 1])
                if t == 4:
                    o_partial([0, 1])
                elif t == 6:
                    o_partial([2])
            pv_and_norm(NT - 1, e_tiles_all[NT - 1])

            # -------- final O partial fused with LayerNorm + store --------
            # LN stats without bn_stats: sum(x) free via the eviction stt's
            # accum_out, sum(x^2) via an ACT Square (ACT is idle in the tail).
            for mq in range(NT):
                o_t = o_tiles[mq]
                ps = scps.tile([128, S], F32, tag="sc", name="o3ps_t")
                for n in range(NCH):
                    pair_mm(
                        ps[:, n * CH:(n + 1) * CH],
                        ctxt[3][:]
                        .rearrange("p (i q) -> p i q", i=2)
                        [:, :, mq * 128:(mq + 1) * 128],
                        wo_t[3][:, :, n * CH:(n + 1) * CH],
                        True, True, PROJ_DR,
                    )
                xsum = stp.tile([128, 1], F32, tag="xsum", name="xsum_t")
                nc.vector.scalar_tensor_tensor(
                    out=o_t[:], in0=ps[:], scalar=1.0 / (WSCALE * CSCALE),
                    in1=o_t[:], op0=ALU.mult, op1=ALU.add,
                    accum_out=xsum[:],
                )
                sq_scr = stp.tile([128, H], F32, tag="sq", name="sq_t")
                xsq = stp.tile([128, 1], F32, tag="xsq", name="xsq_t")
                nc.scalar.activation(
                    sq_scr[:], o_t[:], AF.Square, accum_out=xsq[:],
                )
                # mu = xsum/H; var = xsq/H - mu^2; rstd = 1/sqrt(var+eps)
                mu = stp.tile([128, 1], F32, tag="mu", name="mu_t")
                nc.vector.tensor_scalar_mul(mu[:], xsum[:], 1.0 / H)
                mu2 = stp.tile([128, 1], F32, tag="mu2", name="mu2_t")
                nc.vector.tensor_mul(mu2[:], mu[:], mu[:])
                var = stp.tile([128, 1], F32, tag="var", name="var_t")
                nc.vector.scalar_tensor_tensor(
                    out=var[:], in0=xsq[:], scalar=1.0 / H, in1=mu2[:],
                    op0=ALU.mult, op1=ALU.subtract,
                )
                std = stp.tile([128, 1], F32, tag="std", name="std_t")
                nc.scalar.activation(std[:], var[:], AF.Sqrt, bias=eps_sb[:])
                rstd = stp.tile([128, 1], F32, tag="rstd", name="rstd_t")
                nc.vector.reciprocal(rstd[:], std[:])
                # (x - mu) * rstd as affine: rstd*x + (-mu*rstd)
                nmur = stp.tile([128, 1], F32, tag="nmur", name="nmur_t")
                nc.vector.tensor_scalar(
                    out=nmur[:], in0=mu[:], scalar1=rstd[:], scalar2=-1.0,
                    op0=ALU.mult, op1=ALU.mult,
                )
                if mq % 2 == 0:
                    nc.scalar.activation(
                        o_t[:], o_t[:], AF.Identity,
                        bias=nmur[:], scale=rstd[:],
                    )
                else:
                    nc.vector.tensor_scalar(
                        out=o_t[:], in0=o_t[:], scalar1=rstd[:], scalar2=nmur[:],
                        op0=ALU.mult, op1=ALU.add,
                    )
                if use_gb:
                    nc.vector.tensor_mul(o_t[:], o_t[:], gamma_sb[:])
                    nc.vector.tensor_add(o_t[:], o_t[:], beta_sb[:])
                (nc.gpsimd if mq % 2 == 0 else nc.sync).dma_start(
                    out=out_d[mq * 128:(mq + 1) * 128, :], in_=o_t
                )

    nc.compile()
    return nc


def _host_prep_fp8(hidden_states, attention_mask, Wq, bq, Wk, bk, Wv, bv,
                   Wo, bo, ln_gamma, ln_beta):
    import ml_dtypes

    f32 = np.float32
    fp8 = ml_dtypes.float8_e4m3
    hs = np.ascontiguousarray(hidden_states, dtype=f32)

    def wpairs(w):
        # W [out, in] -> W^T * WSCALE as [128, 4, 2, out] fp8 pair layout
        wt = np.asarray(w, f32).T * WSCALE
        return np.ascontiguousarray(
            wt.reshape(4, 2, 128, H).transpose(0, 2, 1, 3)).astype(fp8)

    wq_p, wk_p, wv_p, wo_p = wpairs(Wq), wpairs(Wk), wpairs(Wv), wpairs(Wo)
    bq_r = np.ascontiguousarray(np.asarray(bq, f32).reshape(8, 128).T)
    bk_r = np.ascontiguousarray(np.asarray(bk, f32).reshape(8, 128).T)
    bv_r = np.ascontiguousarray(np.asarray(bv, f32).reshape(1, H))
    gamma_r = np.ascontiguousarray(np.asarray(ln_gamma, f32).reshape(1, H))
    beta_r = np.ascontiguousarray(np.asarray(ln_beta, f32).reshape(1, H))
    bo_r = np.asarray(bo, f32)
    mask = np.asarray(attention_mask, f32).reshape(B, S)

    in_maps = []
    for b in range(B):
        xt = hs[b].T  # [H, S]
        xtp = np.ascontiguousarray(
            xt.reshape(4, 2, 128, S).transpose(0, 2, 1, 3)).astype(fp8)
        xr = np.ascontiguousarray(hs[b] + bo_r[None, :])
        mask_r = np.ascontiguousarray(mask[b].reshape(8, 128).T)
        in_maps.append({
            "xtp": xtp, "xr": xr,
            "wqp": wq_p, "wkp": wk_p, "wvp": wv_p, "wop": wo_p,
            "bq": bq_r, "bk": bk_r, "bv": bv_r,
            "mask": mask_r, "gamma": gamma_r, "beta": beta_r,
        })
    return in_maps


def _build(mm_dtype, n_reps=1, use_gb=True):
    if mm_dtype == "fp8":
        return _build_fp8(n_reps, use_gb)
    import concourse.tile as tile
    from concourse import bacc, mybir

    F32 = mybir.dt.float32
    AF = mybir.ActivationFunctionType
    ALU = mybir.AluOpType

    if mm_dtype == "f32":
        DT = F32
        DRAM_DT = F32
    elif mm_dtype == "f32r":
        DT = mybir.dt.float32r
        DRAM_DT = F32  # declare f32, bitcast APs at DMA time
    elif mm_dtype == "bf16":
        DT = mybir.dt.bfloat16
        DRAM_DT = mybir.dt.bfloat16
    else:
        raise ValueError(mm_dtype)

    def dma_in(out_ap, in_ap, eng=None):
        # DMA into a DT-typed tile; for f32r the DRAM side is f32 and we
        # bitcast the source AP (value-preserving; verified on HW).
        if eng is None:
            eng = nc.sync
        if mm_dtype == "f32r":
            in_ap = in_ap.bitcast(DT)
        eng.dma_start(out=out_ap, in_=in_ap)

    nc = bacc.Bacc("TRN2", target_bir_lowering=False)

    xt_d = nc.dram_tensor("xt", [H, S], DRAM_DT, kind="ExternalInput")
    xr_d = nc.dram_tensor("xr", [S, H], F32, kind="ExternalInput")
    wq_d = nc.dram_tensor("wq", [H, H], DRAM_DT, kind="ExternalInput")
    wk_d = nc.dram_tensor("wk", [H, H], DRAM_DT, kind="ExternalInput")
    wv_d = nc.dram_tensor("wv", [H, H], DRAM_DT, kind="ExternalInput")
    wo_d = nc.dram_tensor("wo", [H, H], DRAM_DT, kind="ExternalInput")
    bq_d = nc.dram_tensor("bq", [128, 8], F32, kind="ExternalInput")
    bk_d = nc.dram_tensor("bk", [128, 8], F32, kind="ExternalInput")
    bv_d = nc.dram_tensor("bv", [1, H], F32, kind="ExternalInput")
    mask_d = nc.dram_tensor("mask", [128, 8], F32, kind="ExternalInput")
    gamma_d = nc.dram_tensor("gamma", [1, H], F32, kind="ExternalInput")
    beta_d = nc.dram_tensor("beta", [1, H], F32, kind="ExternalInput")
    out_d = nc.dram_tensor("out", [S, H], F32, kind="ExternalOutput")

    NT = 8          # 128-row tiles per 1024 dim
    NCH = 2         # 512-col chunks per 1024 dim
    CH = 512

    with tile.TileContext(nc) as tc:
      for _rep in range(n_reps):
        with (
            tc.tile_pool(name="consts", bufs=1) as cp,
            tc.tile_pool(name="qt", bufs=8) as qt_pool,
            tc.tile_pool(name="kt", bufs=8) as kt_pool,
            tc.tile_pool(name="vt", bufs=8) as vt_pool,
        ):
            bq_sb = cp.tile([128, 8], F32)
            bk_sb = cp.tile([128, 8], F32)
            mask_sb = cp.tile([128, 8], F32)
            nc.sync.dma_start(out=bq_sb, in_=bq_d[:])
            nc.sync.dma_start(out=bk_sb, in_=bk_d[:])
            nc.sync.dma_start(out=mask_sb, in_=mask_d[:])
            bv_row = cp.tile([1, H], F32)
            gamma_row = cp.tile([1, H], F32)
            beta_row = cp.tile([1, H], F32)
            nc.sync.dma_start(out=bv_row, in_=bv_d[:])
            nc.sync.dma_start(out=gamma_row, in_=gamma_d[:])
            nc.sync.dma_start(out=beta_row, in_=beta_d[:])
            bv_sb = cp.tile([128, H], F32)
            gamma_sb = cp.tile([128, H], F32)
            beta_sb = cp.tile([128, H], F32)
            nc.gpsimd.partition_broadcast(bv_sb[:], bv_row[:])
            nc.gpsimd.partition_broadcast(gamma_sb[:], gamma_row[:])
            nc.gpsimd.partition_broadcast(beta_sb[:], beta_row[:])
            eps_sb = cp.tile([128, 1], F32)
            nc.vector.memset(eps_sb[:], LN_EPS)
            ones_sb = cp.tile([128, NH], F32)
            nc.vector.memset(ones_sb[:], 1.0)

            qt = [qt_pool.tile([128, S], DT, tag="qt", name=f"qt{t}") for t in range(NT)]
            kt = [kt_pool.tile([128, S], DT, tag="kt", name=f"kt{t}") for t in range(NT)]
            # v tiles: per k-tile, 16 heads x (64 v-cols + ones col)
            vt = [vt_pool.tile([128, NH * 65], DT, tag="vt", name=f"vt{t}") for t in range(NT)]

            # ---------------- QKV projections ----------------
            with (
                tc.tile_pool(name="xt", bufs=8) as xt_pool,
                tc.tile_pool(name="wp", bufs=13) as wp,
                tc.tile_pool(name="pp", bufs=4, space="PSUM") as pp,
            ):
                xt = []
                for t in range(NT):
                    x_t = xt_pool.tile([128, S], DT, tag="xt", name=f"xt{t}")
                    dma_in(x_t, xt_d[t * 128:(t + 1) * 128, :],
                           eng=(nc.scalar if t % 2 == 0 else nc.gpsimd))
                    xt.append(x_t)

                # V projection: natural [k, dv] layout; lhsT = XT tiles.
                wv_tiles = []
                for t in range(NT):
                    w_t = wp.tile([128, H], DT, tag="w", name=f"w_v{t}")
                    dma_in(w_t, wv_d[t * 128:(t + 1) * 128, :],
                           eng=(nc.sync if t % 2 == 0 else nc.scalar))
                    wv_tiles.append(w_t)
                for mk in range(NT):
                    # ones columns for the softmax-denominator rows
                    nc.vector.tensor_copy(
                        vt[mk][:].rearrange("p (g e) -> p g e", e=65)[:, :, 64:65],
                        ones_sb[:].rearrange("p (g e) -> p g e", e=1),
                    )
                    for n in range(NCH):
                        ps = pp.tile([128, CH], F32, tag="pp", name="pp_t")
                        for h in range(NT):
                            nc.tensor.matmul(
                                ps[:],
                                lhsT=xt[h][:, mk * 128:(mk + 1) * 128],
                                rhs=wv_tiles[h][:, n * CH:(n + 1) * CH],
                                start=(h == 0),
                                stop=(h == NT - 1),
                            )
                        nc.vector.tensor_add(
                            vt[mk][:, n * 8 * 65:(n + 1) * 8 * 65]
                            .rearrange("p (g e) -> p g e", e=65)[:, :, 0:64],
                            ps[:].rearrange("p (g e) -> p g e", e=64),
                            bv_sb[:, n * CH:(n + 1) * CH]
                            .rearrange("p (g e) -> p g e", e=64),
                        )

                for name, w_dram, b_sb, dst in (
                    ("q", wq_d, bq_sb, qt),
                    ("k", wk_d, bk_sb, kt),
                ):
                    w_tiles = []
                    for t in range(NT):
                        w_t = wp.tile([128, H], DT, tag="w", name=f"w_{name}{t}")
                        dma_in(w_t, w_dram[t * 128:(t + 1) * 128, :])
                        w_tiles.append(w_t)
                    for m in range(NT):
                        for n in range(NCH):
                            ps = pp.tile([128, CH], F32, tag="pp", name="pp_t")
                            for h in range(NT):
                                nc.tensor.matmul(
                                    ps[:],
                                    lhsT=w_tiles[h][:, m * 128:(m + 1) * 128],
                                    rhs=xt[h][:, n * CH:(n + 1) * CH],
                                    start=(h == 0),
                                    stop=(h == NT - 1),
                                )
                            nc.vector.tensor_scalar_add(
                                dst[m][:, n * CH:(n + 1) * CH], ps[:],
                                b_sb[:, m:m + 1],
                            )

            # ---------------- attention (per head pair) ----------------
            with (
                tc.tile_pool(name="ep", bufs=12) as ep,
                tc.tile_pool(name="rp", bufs=4) as rp,
                tc.tile_pool(name="rbp", bufs=3) as rbp,
                tc.tile_pool(name="scps", bufs=2, space="PSUM") as scps,
                tc.tile_pool(name="cxps", bufs=4, space="PSUM") as cxps,
            ):
                ctxt = []
                for t in range(NT):  # head pair t = heads 2t, 2t+1
                    ctx_t = qt_pool.tile([128, S], DT, tag="qt", name=f"ctx{t}")
                    ctxt.append(ctx_t)
                    # 4 live PV accumulators: (head, chunk)
                    cxs = [[cxps.tile([65, CH], F32, tag="cx", name="cx_t")
                            for _ in range(NCH)] for _ in range(2)]
                    for k in range(NT):
                        # one [128, S] score psum per head per k-tile (2 banks);
                        # the two q-chunks fill its halves; one exp covers both.
                        # Head A (rows 0-63) and head B (rows 64-127) matmuls are
                        # emitted adjacently per chunk: disjoint PE row groups run
                        # concurrently (row tiling).
                        scs = []
                        for hh in range(2):
                            sc = scps.tile([128, S], F32, tag="sc", name="sc_t")
                            scs.append(sc)
                        for n in range(NCH):
                            for hh in range(2):
                                p0 = hh * 64
                                nc.tensor.matmul(
                                    scs[hh][:, n * CH:(n + 1) * CH],
                                    lhsT=kt[t][p0:p0 + 64, k * 128:(k + 1) * 128],
                                    rhs=qt[t][p0:p0 + 64, n * CH:(n + 1) * CH],
                                    start=True,
                                    stop=True,
                                )
                        for hh in range(2):
                            g = 2 * t + hh
                            e_t = ep.tile([128, S], DT, tag="e", name="e_t")
                            nc.scalar.activation(
                                e_t[:], scs[hh][:], AF.Exp,
                                bias=mask_sb[:, k:k + 1], scale=0.125,
                            )
                            for n in range(NCH):
                                nc.tensor.matmul(
                                    cxs[hh][n][:],
                                    lhsT=vt[k][:, g * 65:(g + 1) * 65],
                                    rhs=e_t[:, n * CH:(n + 1) * CH],
                                    start=(k == 0),
                                    stop=(k == NT - 1),
                                )
                    for hh in range(2):
                        for n in range(NCH):
                            cx = cxs[hh][n]
                            recip = rp.tile([1, CH], F32, tag="recip", name="recip_t")
                            nc.vector.reciprocal(recip[:], cx[64:65, :])
                            rb = rbp.tile([64, CH], F32, tag="rb", name="rb_t")
                            nc.gpsimd.partition_broadcast(rb[:], recip[:])
                            nc.vector.tensor_mul(
                                ctx_t[hh * 64:hh * 64 + 64, n * CH:(n + 1) * CH],
                                cx[0:64, :],
                                rb[:],
                            )

            # ---------------- output proj + residual + LayerNorm ----------------
            with (
                tc.tile_pool(name="wo", bufs=8) as wop,
                tc.tile_pool(name="xr", bufs=5) as xrp,
                tc.tile_pool(name="ob", bufs=4) as obp,
                tc.tile_pool(name="st", bufs=4) as stp,
                tc.tile_pool(name="po", bufs=4, space="PSUM") as po,
            ):
                wo_tiles = []
                for t in range(NT):
                    w_t = wop.tile([128, H], DT, tag="wo", name=f"wo{t}")
                    dma_in(w_t, wo_d[t * 128:(t + 1) * 128, :])
                    wo_tiles.append(w_t)
                for mq in range(NT):
                    xr_t = xrp.tile([128, H], F32, tag="xr", name="xr_t")
                    (nc.sync if mq % 2 == 0 else nc.gpsimd).dma_start(
                        out=xr_t, in_=xr_d[mq * 128:(mq + 1) * 128, :]
                    )
                    o_t = obp.tile([128, H], F32, tag="ob", name="ob_t")
                    for n in range(NCH):
                        ps = po.tile([128, CH], F32, tag="po", name="po_t")
                        for t in range(NT):
                            nc.tensor.matmul(
                                ps[:],
                                lhsT=ctxt[t][:, mq * 128:(mq + 1) * 128],
                                rhs=wo_tiles[t][:, n * CH:(n + 1) * CH],
                                start=(t == 0),
                                stop=(t == NT - 1),
                            )
                        nc.vector.tensor_add(
                            o_t[:, n * CH:(n + 1) * CH], ps[:],
                            xr_t[:, n * CH:(n + 1) * CH],
                        )
                    stats = stp.tile([128, 2, 6], F32, tag="stats", name="stats_t")
                    for sg in range(2):
                        nc.vector.bn_stats(
                            stats[:, sg, :], o_t[:, sg * CH:(sg + 1) * CH]
                        )
                    mv = stp.tile([128, 2], F32, tag="mv", name="mv_t")
                    nc.vector.bn_aggr(mv[:], stats[:])
                    mu = mv[:, 0:1]
                    var = mv[:, 1:2]
                    std = stp.tile([128, 1], F32, tag="std", name="std_t")
                    nc.scalar.activation(std[:], var[:], AF.Sqrt, bias=eps_sb[:])
                    rstd = stp.tile([128, 1], F32, tag="rstd", name="rstd_t")
                    nc.vector.reciprocal(rstd[:], std[:])
                    # (x - mu) * rstd as ACT affine: rstd*x + (-mu*rstd)
                    nmur = stp.tile([128, 1], F32, tag="nmur", name="nmur_t")
                    nc.vector.tensor_scalar(
                        out=nmur[:], in0=mu, scalar1=rstd[:], scalar2=-1.0,
                        op0=ALU.mult, op1=ALU.mult,
                    )
                    nc.scalar.activation(
                        o_t[:], o_t[:], AF.Identity,
                        bias=nmur[:], scale=rstd[:],
                    )
                    if use_gb:
                        nc.vector.tensor_mul(o_t[:], o_t[:], gamma_sb[:])
                        nc.vector.tensor_add(o_t[:], o_t[:], beta_sb[:])
                    (nc.gpsimd if mq % 2 == 0 else nc.sync).dma_start(
                        out=out_d[mq * 128:(mq + 1) * 128, :], in_=o_t
                    )

    nc.compile()
    return nc


def _host_prep(mm_dtype, hidden_states, attention_mask, Wq, bq, Wk, bk, Wv, bv,
               Wo, bo, ln_gamma, ln_beta):
    if mm_dtype == "fp8":
        return _host_prep_fp8(hidden_states, attention_mask, Wq, bq, Wk, bk,
                              Wv, bv, Wo, bo, ln_gamma, ln_beta)
    f32 = np.float32
    hs = np.ascontiguousarray(hidden_states, dtype=f32)
    if mm_dtype == "bf16":
        import ml_dtypes
        wdt = ml_dtypes.bfloat16
    else:
        wdt = f32
    wqT = np.ascontiguousarray(np.asarray(Wq, dtype=f32).T).astype(wdt)
    wkT = np.ascontiguousarray(np.asarray(Wk, dtype=f32).T).astype(wdt)
    wvT = np.ascontiguousarray(np.asarray(Wv, dtype=f32).T).astype(wdt)
    woT = np.ascontiguousarray(np.asarray(Wo, dtype=f32).T).astype(wdt)
    bq_r = np.ascontiguousarray(np.asarray(bq, f32).reshape(8, 128).T)
    bk_r = np.ascontiguousarray(np.asarray(bk, f32).reshape(8, 128).T)
    bv_r = np.ascontiguousarray(np.asarray(bv, f32).reshape(1, H))
    gamma_r = np.ascontiguousarray(np.asarray(ln_gamma, f32).reshape(1, H))
    beta_r = np.ascontiguousarray(np.asarray(ln_beta, f32).reshape(1, H))
    bo_r = np.asarray(bo, f32)
    mask = np.asarray(attention_mask, f32).reshape(B, S)

    in_maps = []
    for b in range(B):
        xt = np.ascontiguousarray(hs[b].T).astype(wdt)
        xr = np.ascontiguousarray(hs[b] + bo_r[None, :])
        mask_r = np.ascontiguousarray(mask[b].reshape(8, 128).T)
        in_maps.append({
            "xt": xt, "xr": xr,
            "wq": wqT, "wk": wkT, "wv": wvT, "wo": woT,
            "bq": bq_r, "bk": bk_r, "bv": bv_r,
            "mask": mask_r, "gamma": gamma_r, "beta": beta_r,
        })
    return in_maps


def get_nc(mm_dtype=MM_DTYPE, n_reps=1, use_gb=True):
    key = (mm_dtype, n_reps, use_gb)
    if key not in _compiled:
        _compiled[key] = _build(mm_dtype, n_reps, use_gb)
    return _compiled[key]


def kernel(hidden_states, attention_mask, Wq, bq, Wk, bk, Wv, bv, Wo, bo,
           ln_gamma, ln_beta):
    from concourse.bass_utils import run_bass_kernel_spmd

    use_gb = not (
        np.all(np.asarray(ln_gamma) == 1.0) and np.all(np.asarray(ln_beta) == 0.0)
    )
    nc = get_nc(MM_DTYPE, use_gb=use_gb)
    in_maps = _host_prep(MM_DTYPE, hidden_states, attention_mask, Wq, bq,
                         Wk, bk, Wv, bv, Wo, bo, ln_gamma, ln_beta)
    res = run_bass_kernel_spmd(nc, in_maps, list(range(N_CORES)))
    out = np.stack([np.asarray(res.results[i]["out"]) for i in range(N_CORES)])
    return out.astype(np.float32)
